# revision 26
# baseline (speedup 1.0000x reference)
"""Soft decision-tree (MoE-routing style) model on 8 Trainium2 NeuronCores.

Computation (see reference):
    d      = sigmoid(x @ W^T)                  x:[B,1024]  W:[1023,1024]
    probs  = level-by-level path products       -> [B, 1024] leaf probs
    out    = softmax(probs @ L, axis=1)         L:[1024,1024]

Strategy (per core, data-parallel over batch):
  * Contraction-on-partitions layout everywhere -> no transposes on device:
      GEMM1: z[slot, b]   = Wp^T-chunks (lhsT) x x^T-chunks (rhs)
      GEMM2: logit[b, o]  = P10-chunks  (lhsT) x L-chunks   (rhs)
  * Host pre-permutes weights:
      - node dim padded 1023 -> 1024 "slots", level l at [2^l, 2^(l+1)),
        little-endian order within the level (tree recursion is pure
        concat, never interleave).
      - leaf predictions permuted by 10-bit bit-reversal to match.
  * GEMM1 runs a single fp32r pass (x, W rounded to fp32r's 12-bit
    mantissa). The input-rounding error reaches ~2.8e-3 max rel err at
    the softmax output (measured against the fp32 reference), well
    inside the 2e-2 gate, so no fp8 correction passes are needed and
    GEMM1 costs exactly one PE pass.
  * Tree levels 0-6 are evaluated in log-space with one fp32 PE matmul
    group against a constant 0/1 selection matrix (M7) so every
    vector-engine op is full-width and partition-aligned.
  * Levels 7-9 are plain full-width f32 mul/sub; the last level writes
    float32r (rounded) because GEMM2 consumes it as the stationary operand.
  * Chunk-0 log terms come straight from softplus on the PSUM z tile
    (ln(sigmoid(z)) = -softplus(-z)); the negation is folded into M7.
    Readiness order then clusters ACT tables (softplus -> sigmoid ->
    exp) to ~3 table loads per block instead of 6.
  * GEMM2+softmax of block k-1 is emitted after GEMM1 of block k
    (1-block software pipeline) to keep the PE busy across block seams.
"""

import numpy as np

import concourse.bacc as bacc
import concourse.bass as bass
import concourse.mybir as mybir
import concourse.tile as tile
from concourse.bass_utils import run_bass_kernel_spmd

AF = mybir.ActivationFunctionType
f32 = mybir.dt.float32
f32r = mybir.dt.float32r
MAX_DEPTH = 10
B = 32768
F = 1024
NOUT = 1024
NLEAF = 1024
NCORES = 8
BL = B // NCORES          # rows per core
BLOCK = 512               # batch columns processed per block
NBLOCKS = BL // BLOCK


def _bitrev(i: int, bits: int) -> int:
    r = 0
    for b in range(bits):
        r = (r << 1) | ((i >> b) & 1)
    return r


def _round_f32r(a: np.ndarray) -> np.ndarray:
    """Round fp32 to fp32r (1s/8e/11m, value held in the top 20 bits), RNE."""
    u = np.ascontiguousarray(a, dtype=np.float32).view(np.uint32)
    lo = u & np.uint32(0xFFF)
    base = u & np.uint32(0xFFFFF000)
    rnd = (lo > 0x800) | ((lo == 0x800) & (((u >> np.uint32(12)) & np.uint32(1)) == 1))
    out = base + (rnd.astype(np.uint32) << np.uint32(12))
    return out.view(np.float32)


def _host_prep(feature_thresholds: np.ndarray, leaf_predictions: np.ndarray):
    """Build the permuted/padded constant tensors."""
    ft = np.asarray(feature_thresholds, dtype=np.float32)
    lp = np.asarray(leaf_predictions, dtype=np.float32)

    # Padded node slots: level l occupies [2^l, 2^(l+1)), little-endian order
    # within the level: slot 2^l + j holds BFS node (2^l - 1) + bitrev_l(j).
    wp = np.zeros((1024, F), dtype=np.float32)
    for lvl in range(MAX_DEPTH):
        n = 1 << lvl
        src = np.fromiter(
            ((n - 1) + _bitrev(j, lvl) for j in range(n)), dtype=np.int64, count=n
        )
        wp[n : 2 * n] = ft[src]
    wt = np.ascontiguousarray(wp.T)  # [F, 1024 slots]
    wt_r = _round_f32r(wt)

    # Leaf predictions in little-endian leaf order.
    perm = np.fromiter(
        (_bitrev(i, MAX_DEPTH) for i in range(NLEAF)), dtype=np.int64, count=NLEAF
    )
    lperm = np.ascontiguousarray(lp[perm])  # [1024, NOUT]

    # M7 selection matrix: logP7[j] = sum over levels 0..6 of ln(d or 1-d).
    # Rows 0..127   multiply ln(sigmoid(z))  of slot s.
    # Rows 128..255 multiply ln(1-sigmoid(z)) of slot s-128.
    m7 = np.zeros((256, 128), dtype=np.float32)
    for j in range(128):
        for lvl in range(7):
            slot = (1 << lvl) + (j & ((1 << lvl) - 1))
            bit = (j >> lvl) & 1
            m7[slot + 128 * bit, j] = 1.0
    return wt_r, _round_f32r(lperm), m7  # m7 is 0/1: exact in fp32r


def _build_program(n_blocks: int = NBLOCKS, block: int = BLOCK) -> bass.Bass:
    nc = bacc.Bacc()
    nb = n_blocks * block
    xt = nc.dram_tensor("xt", [F, nb], f32r, kind="ExternalInput")
    wt = nc.dram_tensor("wt", [F, 1024], f32r, kind="ExternalInput")
    lp = nc.dram_tensor("lp", [NLEAF, NOUT], f32r, kind="ExternalInput")
    m7 = nc.dram_tensor("m7", [256, 128], f32r, kind="ExternalInput")
    out = nc.dram_tensor("out", [nb, NOUT], f32, kind="ExternalOutput")

    with tile.TileContext(nc) as tc:
        with (
            tc.tile_pool(name="consts", bufs=1) as consts,
            tc.tile_pool(name="xtp", bufs=2) as xtp,
            tc.tile_pool(name="dp", bufs=2) as dp,
            tc.tile_pool(name="sgp", bufs=1) as sgp,
            tc.tile_pool(name="lnfull", bufs=1) as lnfull,
            tc.tile_pool(name="tree89", bufs=1) as tree89,
            tc.tile_pool(name="p10pool", bufs=2) as p10pool,
            tc.tile_pool(name="outp", bufs=2) as outp,
            tc.tile_pool(name="smalls", bufs=2) as smalls,
            tc.tile_pool(name="zps", bufs=2, space="PSUM") as zps,
            tc.tile_pool(name="p7ps", bufs=2, space="PSUM") as p7ps,
            tc.tile_pool(name="gps", bufs=2, space="PSUM") as gps,
        ):
            # All GEMM1 weights split per node-chunk and interleaved on the
            # HWDGE ring so each accumulation group's weights arrive just
            # ahead of first use (SWDGE is descriptor-rate-limited; avoid).
            wt_re = wt.rearrange("(c p) n -> p c n", p=128)
            wt_sb_n = []
            for k in range(8):
                t = consts.tile([128, 8, 128], f32r, tag=f"wt{k}")
                wt_sb_n.append(t)
            nc.sync.dma_start(out=wt_sb_n[1], in_=wt_re[:, :, 128:256])

            def emit_g2_softmax(st):
                bs_prev, p10_prev = st
                for sb in range(block // 128):
                    g = gps.tile([128, 1024], f32, tag="g")
                    for h in range(2):
                        # accumulation order = p10 production order
                        for i, lc in enumerate((0, 4, 1, 5, 2, 6, 3, 7)):
                            nc.tensor.matmul(
                                g[:, h * 512 : (h + 1) * 512],
                                lhsT=p10_prev[lc][:, sb * 128 : (sb + 1) * 128],
                                rhs=lp_sb[:, lc, h * 512 : (h + 1) * 512],
                                start=(i == 0),
                                stop=(i == 7),
                            )
                    out_t = outp.tile([128, NOUT], f32, tag="out")
                    stot = smalls.tile([128, 1], f32, tag="stot")
                    # |logits| <= max|leaf_pred| (convex combination): exp is
                    # overflow-safe without a max-subtraction pass.
                    nc.scalar.activation(
                        out=out_t, in_=g, func=AF.Exp, accum_out=stot
                    )
                    rcp = smalls.tile([128, 1], f32, tag="rcp")
                    nc.vector.reciprocal(rcp, stot)
                    nc.vector.tensor_scalar_mul(out_t, out_t, rcp)
                    nc.sync.dma_start(
                        out=out[bs_prev + sb * 128 : bs_prev + (sb + 1) * 128, :],
                        in_=out_t,
                    )

            def load_block(bi, split=1):
                bs = bi * block
                xt_sb = xtp.tile([128, 8, block], f32r, tag="xt")
                src = xt[:, bs : bs + block].rearrange("(c p) b -> p c b", p=128)
                step = 8 // split
                for s in range(split):
                    nc.sync.dma_start(
                        out=xt_sb[:, s * step : (s + 1) * step, :],
                        in_=src[:, s * step : (s + 1) * step, :],
                    )
                return (xt_sb,)

            pending = None
            # Block 0 in quarters: the first GEMM1 matmuls start once the
            # first fc pair lands instead of waiting for the full 2 MB.
            tiles = load_block(0, split=4)
            # GEMM1 consumes node chunks in order 1..7 then 0 — preload in
            # the same order (chunk 1 went ahead of the x block above).
            for k in list(range(2, 8)) + [0]:
                nc.sync.dma_start(
                    out=wt_sb_n[k], in_=wt_re[:, :, k * 128 : (k + 1) * 128]
                )
            # lp (4 MB) rides the SAME sync ring after the startup-critical
            # loads: on its own ring it would steal ~half the HBM bandwidth
            # exactly when wt/xt gate the first matmul. Not consumed until
            # the first GEMM2, ~30 us in.
            lp_sb = consts.tile([128, 8, NOUT], f32r)
            nc.sync.dma_start(out=lp_sb, in_=lp.rearrange("(c p) o -> p c o", p=128))
            m7_sb = consts.tile([128, 2, 128], f32r)
            nc.gpsimd.dma_start(out=m7_sb, in_=m7.rearrange("(c p) j -> p c j", p=128))
            ln_eps = consts.tile([128, 1], f32)
            nc.vector.memset(ln_eps, 1e-37)
            for bi in range(n_blocks):
                (xt_sb,) = tiles

                # ---- GEMM1: z[slot, b] = one fp32r pass. Chunk 0 (the
                # log-space levels) is computed LAST so the ACT readiness
                # order is sigmoid x8 -> ln -> exp: 3 table loads/block. ----
                zpsums = {}
                for nch in list(range(1, 8)) + [0]:
                    zp = zps.tile([128, block], f32, tag="z")
                    for fc in range(8):
                        nc.tensor.matmul(
                            zp,
                            lhsT=wt_sb_n[nch][:, fc, :],
                            rhs=xt_sb[:, fc, :],
                            start=(fc == 0), stop=(fc == 7),
                        )
                    zpsums[nch] = zp

                # ---- ACT: sigmoids for the direct levels 7-9, then the
                # chunk-0 sigmoid+ln; ln(1-d) comes from the identity
                # ln(1-sigmoid(z)) = ln(sigmoid(z)) - z on the DVE (min 0
                # guards the sigmoid-underflow regime z < -87). ----
                dcs = {}
                for nch in range(1, 8):
                    d = dp.tile([128, block], f32, tag=f"dc{nch}")
                    nc.scalar.activation(out=d, in_=zpsums[nch], func=AF.Sigmoid)
                    dcs[nch] = d
                sg_pos = sgp.tile([128, block], f32, tag="sgpos")
                nc.scalar.activation(out=sg_pos, in_=zpsums[0], func=AF.Sigmoid)
                # ln(d) at FULL f32: the subtraction below needs exact
                # cancellation for large negative z (f32r-rounding lnf
                # first would leave a |z|*2^-12 absolute error on the live
                # 1-d branch). min(.,0) is an exact no-op on ln values and
                # rounds to f32r for the M7 matmul operands.
                lnf32 = sgp.tile([128, block], f32, tag="lnf32")
                nc.scalar.activation(out=lnf32, in_=sg_pos, func=AF.Ln, bias=ln_eps)
                lnf_p = lnfull.tile([128, block], f32r, tag="lnfp")
                nc.vector.tensor_scalar_min(lnf_p, lnf32, 0.0)
                lnf_n = sgp.tile([128, block], f32r, tag="lnfn")
                nc.vector.tensor_sub(lnf_n, lnf32, zpsums[0])
                nc.vector.tensor_scalar_min(lnf_n, lnf_n.bitcast(f32), 0.0)

                if bi + 1 < n_blocks:
                    tiles = load_block(bi + 1)

                # ---- GEMM2 + softmax of the previous block (PE fills the
                # gap while ACT/DVE work through this block's tree) ----
                if pending is not None:
                    emit_g2_softmax(pending)

                # ---- levels 0-6 in log space on the PE (fp32r: the ln
                # rounding is ~1.2e-4 relative, well inside the error floor) ----
                lp7 = p7ps.tile([128, block], f32, tag="logp7")
                nc.tensor.matmul(lp7, lhsT=m7_sb[:, 0, :], rhs=lnf_p, start=True, stop=False)
                nc.tensor.matmul(lp7, lhsT=m7_sb[:, 1, :], rhs=lnf_n, start=False, stop=True)
                p7 = tree89.tile([128, block], f32, tag="p9_0")
                nc.scalar.activation(out=p7, in_=lp7, func=AF.Exp)

                # ---- levels 7-9, all full-width partition-aligned f32 ----
                p8a = tree89.tile([128, block], f32, tag="p8a")
                nc.vector.tensor_mul(p8a, p7, dcs[1])
                p8b = tree89.tile([128, block], f32, tag="p8b")
                nc.vector.tensor_sub(p8b, p7, p8a)

                p9 = []
                t = tree89.tile([128, block], f32, tag="p9_0")
                nc.vector.tensor_mul(t, p8a, dcs[2])
                p9.append(t)
                t = tree89.tile([128, block], f32, tag="p9_1")
                nc.vector.tensor_mul(t, p8b, dcs[3])
                p9.append(t)
                # in-place: p8a/p8b become p9_2/p9_3
                nc.vector.tensor_sub(p8a, p8a, p9[0])
                p9.append(p8a)
                nc.vector.tensor_sub(p8b, p8b, p9[1])
                p9.append(p8b)

                # level 9 writes fp32r (GEMM2 stationary operand).
                # mul/sub interleaved so p10 lands in the order GEMM2's
                # accumulation consumes it (0,4,1,5,...): the last block's
                # GEMM2 then trails the DVE by ~2 ops instead of ~9.
                p10 = [None] * 8
                for k in range(4):
                    t = p10pool.tile([128, block], f32r, tag=f"p10_{k}")
                    nc.vector.tensor_mul(t, p9[k], dcs[4 + k])
                    p10[k] = t
                    t = p10pool.tile([128, block], f32r, tag=f"p10_{4 + k}")
                    nc.vector.tensor_sub(t, p9[k], p10[k].bitcast(f32))
                    p10[4 + k] = t

                pending = (bi * block, p10)

            emit_g2_softmax(pending)
    nc.finalize()
    return nc


_PROGRAM_CACHE: dict = {}


def _get_program(n_blocks: int = NBLOCKS, block: int = BLOCK) -> bass.Bass:
    key = (n_blocks, block)
    if key not in _PROGRAM_CACHE:
        _PROGRAM_CACHE[key] = _build_program(n_blocks, block)
    return _PROGRAM_CACHE[key]


def kernel(x, feature_thresholds, leaf_predictions, _trace=False):
    x = np.asarray(x, dtype=np.float32)
    wt_r, lperm, m7 = _host_prep(feature_thresholds, leaf_predictions)
    xt = np.ascontiguousarray(x.T)  # [F, B]
    xt_r = _round_f32r(xt)

    nc = _get_program()
    in_maps = []
    for c in range(NCORES):
        in_maps.append(
            {
                "xt": np.ascontiguousarray(xt_r[:, c * BL : (c + 1) * BL]),
                "wt": wt_r,
                "lp": lperm,
                "m7": m7,
            }
        )

    res = run_bass_kernel_spmd(nc, in_maps, core_ids=list(range(NCORES)), trace=_trace)
    out = np.concatenate([res.results[c]["out"] for c in range(NCORES)], axis=0)
    if _trace:
        return out, res
    return out

